# revision 6
# baseline (speedup 1.0000x reference)
"""BEV voxel-pooling (segment_reduce) kernel for 8 Trainium2 NeuronCores.

Strategy (v2: row-aligned accumulation — no one-hot, no scatter)
----------------------------------------------------------------
Host (numpy, cheap — driven only by the small geometry inputs):
  * compute each point's BEV rank (bin id) exactly as the reference does
  * per sample: split each rank's point list into pseudo-segments of at
    most L=32 points; sort pseudo-segments by size (desc) and deal them
    round-robin onto the sample's 4 cores (8 cores total for B=2)
  * per core: group its pseudo-segments (still size-desc) into blocks of
    128; block b needs K_b = size of its largest member chunks. Assign
    pseudo-segment j to PSUM partition row j%128 of block j//128.
  * pack features into chunks: chunk (b, k) partition p holds the k-th
    point of pseudo-segment (b, p) as fp16, or zeros past its size. The
    per-block chunk counts K_b are envelope-maxed across all 8 cores so
    a single SPMD program serves every core (zero rows add nothing).

Device (per core, one SPMD Bass/Tile program):
  * load a 128x128 fp16 identity into the PE once (explicit ldweights;
    every matmul is marked non-self-loading), then for each chunk run
    matmul(psum_block, I, chunk) with start on k==0 / stop on k==K_b-1:
    PSUM partition p of block b accumulates the f32 sum of pseudo-seg
    (b, p). No DVE work at all; the whole run is feature-DMA bound.
  * when block b completes, ACT-copy its [128, 64] PSUM window to SBUF
    (fp16) and DMA it to out[b*128:(b+1)*128] — dense rows, no scatter.

Host gather: out row j of a core is pseudo-segment j's sum; np.add.at
into the (B, 40000, 64) grid by each pseudo-segment's rank, reshape to
the reference layout (B, C, X, Y).
"""
import sys
sys.path.insert(0, '/opt/trn_rl_repo')

import numpy as np

# ---------------- problem constants (hardcoded per spec) ----------------
B, N, C = 2, 6, 64
H_IMG, W_IMG = 256, 704
DS = 16
DSH, DSW = H_IMG // DS, W_IMG // DS          # 16, 44
D0, D1 = 4, 45                                # depth bins -> D = 41
X, Y, Z = 200, 200, 1
NBINS = X * Y * Z                             # 40000
NP_SAMPLE = N * (D1 - D0) * DSH * DSW         # 173184
NCORES = 8
SHARDS_PER_SAMPLE = 4

L = 32        # max points per pseudo-segment
BC = 32       # chunks per feature DMA batch

_compiled = {}


# ---------------- host geometry (matches reference numerics) ----------------
def _compute_ranks(frustum, post_trans, post_rots, intrinsics, extrinsics,
                   bev_res, bev_start_pos):
    frustum = np.asarray(frustum, np.float32)
    post_trans = np.asarray(post_trans, np.float32)
    post_rots = np.asarray(post_rots, np.float32)
    intrinsics = np.asarray(intrinsics, np.float32)
    extrinsics = np.asarray(extrinsics, np.float32)
    bev_res = np.asarray(bev_res, np.float32)
    bev_start_pos = np.asarray(bev_start_pos, np.float32)

    ext_inv = np.linalg.inv(extrinsics.astype(np.float64)).astype(np.float32)
    rot = ext_inv[..., :3, :3]
    trans = ext_inv[..., :3, 3]
    pts = frustum[None, None] - post_trans[:, :, None, None, None, :]
    pr_inv = np.linalg.inv(post_rots.astype(np.float64)).astype(np.float32)
    pts = np.einsum('bnij,bndhwj->bndhwi', pr_inv, pts).astype(np.float32)
    pts = np.concatenate([pts[..., :2] * pts[..., 2:3], pts[..., 2:3]], axis=-1)
    comb = (rot @ np.linalg.inv(intrinsics.astype(np.float64)).astype(np.float32)
            ).astype(np.float32)
    pts = np.einsum('bnij,bndhwj->bndhwi', comb, pts).astype(np.float32)
    geom = pts + trans[:, :, None, None, None, :]

    coords = (geom - (bev_start_pos - bev_res / 2.0)) / bev_res
    ci = coords.reshape(B, -1, 3).astype(np.int32)
    mask = ((ci[..., 0] >= 0) & (ci[..., 0] < X) &
            (ci[..., 1] >= 0) & (ci[..., 1] < Y) &
            (ci[..., 2] >= 0) & (ci[..., 2] < Z))
    rank = ci[..., 0] * (Y * Z) + ci[..., 1] * Z + ci[..., 2]
    return rank, mask


# ---------------- host planning ----------------
def _plan_cores(rank, mask):
    """Split every sample's ranks into <=L-point pseudo-segments, deal them
    round-robin (by desc size) onto 4 cores each; compute the cross-core
    block profile (NB, K_b)."""
    cores = []
    for b in range(B):
        r = rank[b]
        m = mask[b]
        valid = np.nonzero(m)[0]
        order = valid[np.argsort(r[valid], kind='stable')]
        rs = r[order]
        newseg = np.r_[True, rs[1:] != rs[:-1]]
        seg_start = np.nonzero(newseg)[0]
        seg_rank = rs[seg_start]
        seg_cnt = np.diff(np.r_[seg_start, len(rs)])
        nseg = len(seg_start)

        npieces = (seg_cnt + L - 1) // L
        piece_seg = np.repeat(np.arange(nseg), npieces)
        piece_off = np.arange(len(piece_seg)) - np.repeat(
            np.cumsum(npieces) - npieces, npieces)
        piece_start = seg_start[piece_seg] + piece_off * L
        piece_cnt = np.minimum(seg_cnt[piece_seg] - piece_off * L, L).astype(np.int64)
        piece_rank = seg_rank[piece_seg]

        po = np.argsort(-piece_cnt, kind='stable')
        for c in range(SHARDS_PER_SAMPLE):
            sel = po[c::SHARDS_PER_SAMPLE]
            cores.append(dict(
                sample=b,
                start=piece_start[sel],
                cnt=piece_cnt[sel],
                rank=piece_rank[sel],
                order=order,
            ))

    NB = max((len(c['cnt']) + 127) // 128 for c in cores)
    Kb = np.ones(NB, np.int64)
    for c in cores:
        cnt = c['cnt']
        for j in range((len(cnt) + 127) // 128):
            Kb[j] = max(Kb[j], int(cnt[j * 128]))
    base = np.concatenate([[0], np.cumsum(Kb)])[:-1]
    NC = int(Kb.sum())
    return cores, NB, Kb, base, NC


def _build_table(core, feats16_b, NB, Kb, base, NC):
    """Per-core packed feature table [128, NC*C] fp16 (partition-major)."""
    tbl = np.zeros((NC, 128, C), np.float16)
    cnt = core['cnt']
    start = core['start']
    order = core['order']
    n = len(cnt)
    if n:
        seg_ids = np.arange(n)
        blk = seg_ids // 128
        row = seg_ids % 128
        tot = int(cnt.sum())
        pt_seg = np.repeat(seg_ids, cnt)
        within = np.arange(tot) - np.repeat(np.cumsum(cnt) - cnt, cnt)
        src = order[np.repeat(start, cnt) + within]
        chunk = base[blk[pt_seg]] + within
        tbl[chunk, row[pt_seg]] = feats16_b[src]
    return np.ascontiguousarray(tbl.transpose(1, 0, 2).reshape(128, NC * C))


# ---------------- device program ----------------
def _build_kernel(NB, Kb, NC):
    import concourse.bass as bass
    import concourse.bacc as bacc
    import concourse.mybir as mybir
    import concourse.tile as tile
    from contextlib import ExitStack

    F32 = mybir.dt.float32
    F16 = mybir.dt.float16

    nc = bacc.Bacc()
    table = nc.dram_tensor("table", [128, NC * C], F16, kind="ExternalInput")
    ident = nc.dram_tensor("ident", [128, 128], F16, kind="ExternalInput")
    out = nc.dram_tensor("out", [NB * 128, C], F16, kind="ExternalOutput")

    with tile.TileContext(nc) as tc, ExitStack() as ctx:
        const = ctx.enter_context(tc.tile_pool(name="const", bufs=1))
        featp = ctx.enter_context(tc.tile_pool(name="feat", bufs=3))
        stagep = ctx.enter_context(tc.tile_pool(name="stage", bufs=4))
        psump = ctx.enter_context(tc.tile_pool(name="psum", bufs=6, space="PSUM"))

        ident_sb = const.tile([128, 128], F16)
        nc.sync.dma_start(ident_sb[:], ident[:])

        nc.tensor.ldweights(ident_sb[:])

        # Batch schedule: tiny leading batches so the first matmuls start
        # as soon as possible, then steady BC-chunk batches.
        sched = []
        pos = 0
        for sz in (2, 2, 4, 8, 16):
            if pos + sz <= NC:
                sched.append((pos, sz))
                pos += sz
        while pos < NC:
            sz = min(BC, NC - pos)
            sched.append((pos, sz))
            pos += sz
        batch_of = {}
        for bi, (p0, sz) in enumerate(sched):
            for t in range(p0, p0 + sz):
                batch_of[t] = (bi, p0)

        t = 0
        feat = None
        cur_batch = -1
        for b in range(NB):
            kb = int(Kb[b])
            accb = psump.tile([128, C], F32, tag="acc")
            for k in range(kb):
                bi, p0 = batch_of[t]
                if bi != cur_batch:
                    sz = sched[bi][1]
                    feat = featp.tile([128, BC * C], F16)
                    nc.sync.dma_start(feat[:, :sz * C],
                                      table[:, p0 * C:(p0 + sz) * C])
                    cur_batch = bi
                nc.tensor.matmul(
                    accb[:], ident_sb[:],
                    feat[:, (t - p0) * C:(t - p0 + 1) * C],
                    start=(k == 0), stop=(k == kb - 1),
                    skip_group_check=True)
                t += 1
            st = stagep.tile([128, C], F16)
            nc.vector.tensor_copy(st[:], accb[:])
            nc.scalar.dma_start(out[b * 128:(b + 1) * 128, :], st[:])
    nc.finalize()
    return nc


# ---------------- entry point ----------------
def kernel(image_feature, post_trans, post_rots, intrinsics, extrinsics,
           frustum, bev_res, bev_start_pos):
    from concourse.bass_utils import run_bass_kernel_spmd
    import os

    rank, mask = _compute_ranks(frustum, post_trans, post_rots, intrinsics,
                                extrinsics, bev_res, bev_start_pos)
    feats16 = np.asarray(image_feature, np.float32).reshape(
        B, NP_SAMPLE, C).astype(np.float16)
    cores, NB, Kb, base, NC = _plan_cores(rank, mask)

    ident = np.eye(128, dtype=np.float16)
    in_maps = [
        {"table": _build_table(c, feats16[c['sample']], NB, Kb, base, NC),
         "ident": ident}
        for c in cores
    ]

    key = (NB, tuple(int(k) for k in Kb), NC)
    if key not in _compiled:
        _compiled[key] = _build_kernel(NB, Kb, NC)
    nc = _compiled[key]

    trace = bool(int(os.environ.get("BEV_TRACE", "0")))
    res = run_bass_kernel_spmd(nc, in_maps, core_ids=list(range(NCORES)),
                               trace=trace,
                               trace_cores=[0] if trace else None)
    if trace and res.exec_time_ns is not None:
        print(f"HW exec time: {res.exec_time_ns} ns")
        kernel.last_exec_time_ns = res.exec_time_ns
        kernel.last_results = res

    grid = np.zeros((B, NBINS, C), np.float32)
    for ci, core in enumerate(cores):
        o = np.asarray(res.results[ci]["out"], np.float32)
        n = len(core['cnt'])
        if n:
            np.add.at(grid[core['sample']], core['rank'], o[:n])
    return np.ascontiguousarray(
        grid.reshape(B, X, Y, C).transpose(0, 3, 1, 2))


# revision 8
# speedup vs baseline: 1.0745x; 1.0745x over previous
"""BEV voxel-pooling (segment_reduce) kernel for 8 Trainium2 NeuronCores.

Strategy (v2: row-aligned accumulation — no one-hot, no scatter)
----------------------------------------------------------------
Host (numpy, cheap — driven only by the small geometry inputs):
  * compute each point's BEV rank (bin id) exactly as the reference does
  * per sample: split each rank's point list into pseudo-segments of at
    most L=32 points; sort pseudo-segments by size (desc) and deal them
    round-robin onto the sample's 4 cores (8 cores total for B=2)
  * per core: group its pseudo-segments (still size-desc) into blocks of
    128; block b needs K_b = size of its largest member chunks. Assign
    pseudo-segment j to PSUM partition row j%128 of block j//128.
  * pack features into chunks: chunk (b, k) partition p holds the k-th
    point of pseudo-segment (b, p) as fp16, or zeros past its size. The
    per-block chunk counts K_b are envelope-maxed across all 8 cores so
    a single SPMD program serves every core (zero rows add nothing).

Device (per core, one SPMD Bass/Tile program):
  * load a 128x128 fp16 identity into the PE once (explicit ldweights;
    every matmul is marked non-self-loading), then for each chunk run
    matmul(psum_block, I, chunk) with start on k==0 / stop on k==K_b-1:
    PSUM partition p of block b accumulates the f32 sum of pseudo-seg
    (b, p). No DVE work at all; the whole run is feature-DMA bound.
  * when block b completes, ACT-copy its [128, 64] PSUM window to SBUF
    (fp16) and DMA it to out[b*128:(b+1)*128] — dense rows, no scatter.

Host gather: out row j of a core is pseudo-segment j's sum; np.add.at
into the (B, 40000, 64) grid by each pseudo-segment's rank, reshape to
the reference layout (B, C, X, Y).
"""
import sys
sys.path.insert(0, '/opt/trn_rl_repo')

import numpy as np

# ---------------- problem constants (hardcoded per spec) ----------------
B, N, C = 2, 6, 64
H_IMG, W_IMG = 256, 704
DS = 16
DSH, DSW = H_IMG // DS, W_IMG // DS          # 16, 44
D0, D1 = 4, 45                                # depth bins -> D = 41
X, Y, Z = 200, 200, 1
NBINS = X * Y * Z                             # 40000
NP_SAMPLE = N * (D1 - D0) * DSH * DSW         # 173184
NCORES = 8
SHARDS_PER_SAMPLE = 4

L = 32        # max points per pseudo-segment
BC = 32       # chunks per feature DMA batch

_compiled = {}


# ---------------- host geometry (matches reference numerics) ----------------
def _compute_ranks(frustum, post_trans, post_rots, intrinsics, extrinsics,
                   bev_res, bev_start_pos):
    frustum = np.asarray(frustum, np.float32)
    post_trans = np.asarray(post_trans, np.float32)
    post_rots = np.asarray(post_rots, np.float32)
    intrinsics = np.asarray(intrinsics, np.float32)
    extrinsics = np.asarray(extrinsics, np.float32)
    bev_res = np.asarray(bev_res, np.float32)
    bev_start_pos = np.asarray(bev_start_pos, np.float32)

    ext_inv = np.linalg.inv(extrinsics.astype(np.float64)).astype(np.float32)
    rot = ext_inv[..., :3, :3]
    trans = ext_inv[..., :3, 3]
    pts = frustum[None, None] - post_trans[:, :, None, None, None, :]
    pr_inv = np.linalg.inv(post_rots.astype(np.float64)).astype(np.float32)
    pts = np.einsum('bnij,bndhwj->bndhwi', pr_inv, pts).astype(np.float32)
    pts = np.concatenate([pts[..., :2] * pts[..., 2:3], pts[..., 2:3]], axis=-1)
    comb = (rot @ np.linalg.inv(intrinsics.astype(np.float64)).astype(np.float32)
            ).astype(np.float32)
    pts = np.einsum('bnij,bndhwj->bndhwi', comb, pts).astype(np.float32)
    geom = pts + trans[:, :, None, None, None, :]

    coords = (geom - (bev_start_pos - bev_res / 2.0)) / bev_res
    ci = coords.reshape(B, -1, 3).astype(np.int32)
    mask = ((ci[..., 0] >= 0) & (ci[..., 0] < X) &
            (ci[..., 1] >= 0) & (ci[..., 1] < Y) &
            (ci[..., 2] >= 0) & (ci[..., 2] < Z))
    rank = ci[..., 0] * (Y * Z) + ci[..., 1] * Z + ci[..., 2]
    return rank, mask


# ---------------- host planning ----------------
def _plan_cores(rank, mask):
    """Split every sample's ranks into <=L-point pseudo-segments, deal them
    round-robin (by desc size) onto 4 cores each; compute the cross-core
    block profile (NB, K_b)."""
    cores = []
    for b in range(B):
        r = rank[b]
        m = mask[b]
        valid = np.nonzero(m)[0]
        order = valid[np.argsort(r[valid], kind='stable')]
        rs = r[order]
        newseg = np.r_[True, rs[1:] != rs[:-1]]
        seg_start = np.nonzero(newseg)[0]
        seg_rank = rs[seg_start]
        seg_cnt = np.diff(np.r_[seg_start, len(rs)])
        nseg = len(seg_start)

        npieces = (seg_cnt + L - 1) // L
        piece_seg = np.repeat(np.arange(nseg), npieces)
        piece_off = np.arange(len(piece_seg)) - np.repeat(
            np.cumsum(npieces) - npieces, npieces)
        piece_start = seg_start[piece_seg] + piece_off * L
        piece_cnt = np.minimum(seg_cnt[piece_seg] - piece_off * L, L).astype(np.int64)
        piece_rank = seg_rank[piece_seg]

        po = np.argsort(-piece_cnt, kind='stable')
        for c in range(SHARDS_PER_SAMPLE):
            sel = po[c::SHARDS_PER_SAMPLE]
            cores.append(dict(
                sample=b,
                start=piece_start[sel],
                cnt=piece_cnt[sel],
                rank=piece_rank[sel],
                order=order,
            ))

    NB = max((len(c['cnt']) + 127) // 128 for c in cores)
    Kb = np.ones(NB, np.int64)
    for c in cores:
        cnt = c['cnt']
        for j in range((len(cnt) + 127) // 128):
            Kb[j] = max(Kb[j], int(cnt[j * 128]))
    base = np.concatenate([[0], np.cumsum(Kb)])[:-1]
    NC = int(Kb.sum())
    return cores, NB, Kb, base, NC


def _build_table(core, feats16_b, NB, Kb, base, NC):
    """Per-core packed feature table [128, NC*C] fp16 (partition-major)."""
    tbl = np.zeros((NC, 128, C), np.float16)
    cnt = core['cnt']
    start = core['start']
    order = core['order']
    n = len(cnt)
    if n:
        seg_ids = np.arange(n)
        blk = seg_ids // 128
        row = seg_ids % 128
        tot = int(cnt.sum())
        pt_seg = np.repeat(seg_ids, cnt)
        within = np.arange(tot) - np.repeat(np.cumsum(cnt) - cnt, cnt)
        src = order[np.repeat(start, cnt) + within]
        chunk = base[blk[pt_seg]] + within
        tbl[chunk, row[pt_seg]] = feats16_b[src]
    return np.ascontiguousarray(tbl.transpose(1, 0, 2).reshape(128, NC * C))


# ---------------- device program ----------------
def _build_kernel(NB, Kb, NC):
    import concourse.bass as bass
    import concourse.bacc as bacc
    import concourse.mybir as mybir
    import concourse.tile as tile
    from contextlib import ExitStack

    F32 = mybir.dt.float32
    F16 = mybir.dt.float16

    nc = bacc.Bacc()
    table = nc.dram_tensor("table", [128, NC * C], F16, kind="ExternalInput")
    ident = nc.dram_tensor("ident", [128, 128], F16, kind="ExternalInput")
    out = nc.dram_tensor("out", [NB * 128, C], F16, kind="ExternalOutput")

    with tile.TileContext(nc) as tc, ExitStack() as ctx:
        const = ctx.enter_context(tc.tile_pool(name="const", bufs=1))
        featp = ctx.enter_context(tc.tile_pool(name="feat", bufs=3))
        stagep = ctx.enter_context(tc.tile_pool(name="stage", bufs=4))
        psump = ctx.enter_context(tc.tile_pool(name="psum", bufs=6, space="PSUM"))

        ident_sb = const.tile([128, 128], F16)
        nc.sync.dma_start(ident_sb[:], ident[:])

        nc.tensor.ldweights(ident_sb[:])

        # Batch schedule: tiny leading batches so the first matmuls start
        # as soon as possible, then steady BC-chunk batches.
        sched = []
        pos = 0
        for sz in (2, 2, 4, 8, 16):
            if pos + sz <= NC:
                sched.append((pos, sz))
                pos += sz
        while pos < NC:
            sz = min(BC, NC - pos)
            sched.append((pos, sz))
            pos += sz
        batch_of = {}
        for bi, (p0, sz) in enumerate(sched):
            for t in range(p0, p0 + sz):
                batch_of[t] = (bi, p0)

        OG = 4            # blocks per output DMA
        t = 0
        feat = None
        st = None
        cur_batch = -1
        for b in range(NB):
            kb = int(Kb[b])
            accb = psump.tile([128, C], F32, tag="acc")
            for k in range(kb):
                bi, p0 = batch_of[t]
                if bi != cur_batch:
                    sz = sched[bi][1]
                    feat = featp.tile([128, BC * C], F16)
                    nc.sync.dma_start(feat[:, :sz * C],
                                      table[:, p0 * C:(p0 + sz) * C])
                    cur_batch = bi
                nc.tensor.matmul(
                    accb[:], ident_sb[:],
                    feat[:, (t - p0) * C:(t - p0 + 1) * C],
                    start=(k == 0), stop=(k == kb - 1),
                    skip_group_check=True)
                t += 1
            if b % OG == 0:
                st = stagep.tile([128, OG * C], F16)
            g = b % OG
            nc.vector.tensor_copy(st[:, g * C:(g + 1) * C], accb[:])
            if g == OG - 1 or b == NB - 1:
                b0 = b - g
                dst = out[b0 * 128:(b + 1) * 128, :].rearrange(
                    "(j p) c -> p j c", p=128)
                src = st[:, :(g + 1) * C].rearrange("p (j c) -> p j c", c=C)
                nc.scalar.dma_start(dst, src)

    # Drop redundant identity reloads: every matmul uses the same stationary
    # weights, so only the first Ldweights must survive. The tile scheduler
    # emits one sync-free Ldweights per matmul (ldweights_flag=False on the
    # Matmult itself); removing them leaves the loaded array untouched.
    for f in nc.m.functions:
        for bb in f.blocks:
            ins = list(bb.instructions)
            seen = False
            keep = []
            removed = 0
            for x in ins:
                if str(x.opcode) == 'Ldweights':
                    si = x.sync_info
                    empty = si is None or (len(si.on_wait) == 0
                                           and len(si.on_update) == 0)
                    if seen and empty:
                        removed += 1
                        continue
                    seen = True
                keep.append(x)
            if removed:
                bb.instructions = keep
    nc.finalize()
    return nc


# ---------------- entry point ----------------
def kernel(image_feature, post_trans, post_rots, intrinsics, extrinsics,
           frustum, bev_res, bev_start_pos):
    from concourse.bass_utils import run_bass_kernel_spmd
    import os

    rank, mask = _compute_ranks(frustum, post_trans, post_rots, intrinsics,
                                extrinsics, bev_res, bev_start_pos)
    feats16 = np.asarray(image_feature, np.float32).reshape(
        B, NP_SAMPLE, C).astype(np.float16)
    cores, NB, Kb, base, NC = _plan_cores(rank, mask)

    ident = np.eye(128, dtype=np.float16)
    in_maps = [
        {"table": _build_table(c, feats16[c['sample']], NB, Kb, base, NC),
         "ident": ident}
        for c in cores
    ]

    key = (NB, tuple(int(k) for k in Kb), NC)
    if key not in _compiled:
        _compiled[key] = _build_kernel(NB, Kb, NC)
    nc = _compiled[key]

    trace = bool(int(os.environ.get("BEV_TRACE", "0")))
    res = run_bass_kernel_spmd(nc, in_maps, core_ids=list(range(NCORES)),
                               trace=trace,
                               trace_cores=[0] if trace else None)
    if trace and res.exec_time_ns is not None:
        print(f"HW exec time: {res.exec_time_ns} ns")
        kernel.last_exec_time_ns = res.exec_time_ns
        kernel.last_results = res

    grid = np.zeros((B, NBINS, C), np.float32)
    for ci, core in enumerate(cores):
        o = np.asarray(res.results[ci]["out"], np.float32)
        n = len(core['cnt'])
        if n:
            np.add.at(grid[core['sample']], core['rank'], o[:n])
    return np.ascontiguousarray(
        grid.reshape(B, X, Y, C).transpose(0, 3, 1, 2))


# revision 12
# speedup vs baseline: 1.4237x; 1.3249x over previous
"""BEV voxel-pooling (segment_reduce) kernel for 8 Trainium2 NeuronCores.

Strategy (v2: row-aligned accumulation — no one-hot, no scatter)
----------------------------------------------------------------
Host (numpy, cheap — driven only by the small geometry inputs):
  * compute each point's BEV rank (bin id) exactly as the reference does
  * per sample: split each rank's point list into pseudo-segments of at
    most L=32 points; sort pseudo-segments by size (desc) and deal them
    round-robin onto the sample's 4 cores (8 cores total for B=2)
  * per core: group its pseudo-segments (still size-desc) into blocks of
    128; block b needs K_b = size of its largest member chunks. Assign
    pseudo-segment j to PSUM partition row j%128 of block j//128.
  * pack features into chunks: chunk (b, k) partition p holds the k-th
    point of pseudo-segment (b, p) as fp16, or zeros past its size. The
    per-block chunk counts K_b are envelope-maxed across all 8 cores so
    a single SPMD program serves every core (zero rows add nothing).

Device (per core, one SPMD Bass/Tile program):
  * load a 128x128 fp16 identity into the PE once (explicit ldweights;
    every matmul is marked non-self-loading), then for each chunk run
    matmul(psum_block, I, chunk) with start on k==0 / stop on k==K_b-1:
    PSUM partition p of block b accumulates the f32 sum of pseudo-seg
    (b, p). No DVE work at all; the whole run is feature-DMA bound.
  * when block b completes, ACT-copy its [128, 64] PSUM window to SBUF
    (fp16) and DMA it to out[b*128:(b+1)*128] — dense rows, no scatter.

Host gather: out row j of a core is pseudo-segment j's sum; np.add.at
into the (B, 40000, 64) grid by each pseudo-segment's rank, reshape to
the reference layout (B, C, X, Y).
"""
import sys
sys.path.insert(0, '/opt/trn_rl_repo')

import numpy as np

# ---------------- problem constants (hardcoded per spec) ----------------
B, N, C = 2, 6, 64
H_IMG, W_IMG = 256, 704
DS = 16
DSH, DSW = H_IMG // DS, W_IMG // DS          # 16, 44
D0, D1 = 4, 45                                # depth bins -> D = 41
X, Y, Z = 200, 200, 1
NBINS = X * Y * Z                             # 40000
NP_SAMPLE = N * (D1 - D0) * DSH * DSW         # 173184
NCORES = 8
SHARDS_PER_SAMPLE = 4

L = 32        # max points per pseudo-segment
BC = 32       # chunks per feature DMA batch

_compiled = {}


# ---------------- host geometry (matches reference numerics) ----------------
def _compute_ranks(frustum, post_trans, post_rots, intrinsics, extrinsics,
                   bev_res, bev_start_pos):
    frustum = np.asarray(frustum, np.float32)
    post_trans = np.asarray(post_trans, np.float32)
    post_rots = np.asarray(post_rots, np.float32)
    intrinsics = np.asarray(intrinsics, np.float32)
    extrinsics = np.asarray(extrinsics, np.float32)
    bev_res = np.asarray(bev_res, np.float32)
    bev_start_pos = np.asarray(bev_start_pos, np.float32)

    ext_inv = np.linalg.inv(extrinsics.astype(np.float64)).astype(np.float32)
    rot = ext_inv[..., :3, :3]
    trans = ext_inv[..., :3, 3]
    pts = frustum[None, None] - post_trans[:, :, None, None, None, :]
    pr_inv = np.linalg.inv(post_rots.astype(np.float64)).astype(np.float32)
    pts = np.einsum('bnij,bndhwj->bndhwi', pr_inv, pts).astype(np.float32)
    pts = np.concatenate([pts[..., :2] * pts[..., 2:3], pts[..., 2:3]], axis=-1)
    comb = (rot @ np.linalg.inv(intrinsics.astype(np.float64)).astype(np.float32)
            ).astype(np.float32)
    pts = np.einsum('bnij,bndhwj->bndhwi', comb, pts).astype(np.float32)
    geom = pts + trans[:, :, None, None, None, :]

    coords = (geom - (bev_start_pos - bev_res / 2.0)) / bev_res
    ci = coords.reshape(B, -1, 3).astype(np.int32)
    mask = ((ci[..., 0] >= 0) & (ci[..., 0] < X) &
            (ci[..., 1] >= 0) & (ci[..., 1] < Y) &
            (ci[..., 2] >= 0) & (ci[..., 2] < Z))
    rank = ci[..., 0] * (Y * Z) + ci[..., 1] * Z + ci[..., 2]
    return rank, mask


# ---------------- host planning ----------------
def _plan_cores(rank, mask):
    """Split every sample's ranks into <=L-point pseudo-segments, deal them
    round-robin (by desc size) onto 4 cores each; compute the cross-core
    block profile (NB, K_b)."""
    cores = []
    for b in range(B):
        r = rank[b]
        m = mask[b]
        valid = np.nonzero(m)[0]
        order = valid[np.argsort(r[valid], kind='stable')]
        rs = r[order]
        newseg = np.r_[True, rs[1:] != rs[:-1]]
        seg_start = np.nonzero(newseg)[0]
        seg_rank = rs[seg_start]
        seg_cnt = np.diff(np.r_[seg_start, len(rs)])
        nseg = len(seg_start)

        npieces = (seg_cnt + L - 1) // L
        piece_seg = np.repeat(np.arange(nseg), npieces)
        piece_off = np.arange(len(piece_seg)) - np.repeat(
            np.cumsum(npieces) - npieces, npieces)
        piece_start = seg_start[piece_seg] + piece_off * L
        piece_cnt = np.minimum(seg_cnt[piece_seg] - piece_off * L, L).astype(np.int64)
        piece_rank = seg_rank[piece_seg]

        po = np.argsort(-piece_cnt, kind='stable')
        for c in range(SHARDS_PER_SAMPLE):
            sel = po[c::SHARDS_PER_SAMPLE]
            cores.append(dict(
                sample=b,
                start=piece_start[sel],
                cnt=piece_cnt[sel],
                rank=piece_rank[sel],
                order=order,
            ))

    NB = max((len(c['cnt']) + 127) // 128 for c in cores)
    Kb = np.ones(NB, np.int64)
    for c in cores:
        cnt = c['cnt']
        for j in range((len(cnt) + 127) // 128):
            Kb[j] = max(Kb[j], int(cnt[j * 128]))
    base = np.concatenate([[0], np.cumsum(Kb)])[:-1]
    NC = int(Kb.sum())
    return cores, NB, Kb, base, NC


def _build_table(core, feats16_b, NB, Kb, base, NC):
    """Per-core packed feature table [128, NC*C] fp16 (partition-major)."""
    tbl = np.zeros((NC, 128, C), np.float16)
    cnt = core['cnt']
    start = core['start']
    order = core['order']
    n = len(cnt)
    if n:
        seg_ids = np.arange(n)
        blk = seg_ids // 128
        row = seg_ids % 128
        tot = int(cnt.sum())
        pt_seg = np.repeat(seg_ids, cnt)
        within = np.arange(tot) - np.repeat(np.cumsum(cnt) - cnt, cnt)
        src = order[np.repeat(start, cnt) + within]
        chunk = base[blk[pt_seg]] + within
        tbl[chunk, row[pt_seg]] = feats16_b[src]
    return np.ascontiguousarray(tbl.transpose(1, 0, 2).reshape(128, NC * C))


# ---------------- device program ----------------
def _build_kernel(NB, Kb, NC):
    import concourse.bass as bass
    import concourse.bacc as bacc
    import concourse.mybir as mybir
    import concourse.tile as tile
    from contextlib import ExitStack

    F32 = mybir.dt.float32
    F16 = mybir.dt.float16

    nc = bacc.Bacc()
    table = nc.dram_tensor("table", [128, NC * C], F16, kind="ExternalInput")
    ident = nc.dram_tensor("ident", [128, 128], F16, kind="ExternalInput")
    out = nc.dram_tensor("out", [NB * 128, C], F16, kind="ExternalOutput")

    with tile.TileContext(nc) as tc, ExitStack() as ctx:
        const = ctx.enter_context(tc.tile_pool(name="const", bufs=1))
        featp = ctx.enter_context(tc.tile_pool(name="feat", bufs=6))
        stagep = ctx.enter_context(tc.tile_pool(name="stage", bufs=4))
        psump = ctx.enter_context(tc.tile_pool(name="psum", bufs=6, space="PSUM"))

        ident_sb = const.tile([128, 128], F16)
        nc.sync.dma_start(ident_sb[:], ident[:])

        nc.tensor.ldweights(ident_sb[:])

        # Batch schedule: tiny leading batches so the first matmuls start
        # as soon as possible, then steady BC-chunk batches.
        sched = []
        pos = 0
        for sz in (4, 8, 16):
            if pos + sz <= NC:
                sched.append((pos, sz))
                pos += sz
        while pos < NC:
            sz = min(BC, NC - pos)
            sched.append((pos, sz))
            pos += sz
        batch_of = {}
        for bi, (p0, sz) in enumerate(sched):
            for t in range(p0, p0 + sz):
                batch_of[t] = (bi, p0)

        OG = 4            # blocks per output DMA
        t = 0
        feat = None
        st = None
        cur_batch = -1
        for b in range(NB):
            kb = int(Kb[b])
            accb = psump.tile([128, C], F32, tag="acc")
            for k in range(kb):
                bi, p0 = batch_of[t]
                if bi != cur_batch:
                    sz = sched[bi][1]
                    feat = featp.tile([128, BC * C], F16)
                    eng = nc.sync if bi % 2 == 0 else nc.scalar
                    eng.dma_start(feat[:, :sz * C],
                                  table[:, p0 * C:(p0 + sz) * C])
                    cur_batch = bi
                nc.tensor.matmul(
                    accb[:], ident_sb[:],
                    feat[:, (t - p0) * C:(t - p0 + 1) * C],
                    start=(k == 0), stop=(k == kb - 1),
                    skip_group_check=True)
                t += 1
            if b % OG == 0:
                st = stagep.tile([128, OG * C], F16)
            g = b % OG
            nc.vector.tensor_copy(st[:, g * C:(g + 1) * C], accb[:])
            if g == OG - 1 or b == NB - 1:
                b0 = b - g
                dst = out[b0 * 128:(b + 1) * 128, :].rearrange(
                    "(j p) c -> p j c", p=128)
                src = st[:, :(g + 1) * C].rearrange("p (j c) -> p j c", c=C)
                nc.gpsimd.dma_start(dst, src)

    # Drop redundant identity reloads: every matmul uses the same stationary
    # weights, so only the first Ldweights must survive. The tile scheduler
    # emits one sync-free Ldweights per matmul (ldweights_flag=False on the
    # Matmult itself); removing them leaves the loaded array untouched.
    for f in nc.m.functions:
        for bb in f.blocks:
            ins = list(bb.instructions)
            seen = False
            keep = []
            removed = 0
            for x in ins:
                if str(x.opcode) == 'Ldweights':
                    si = x.sync_info
                    empty = si is None or (len(si.on_wait) == 0
                                           and len(si.on_update) == 0)
                    if seen and empty:
                        removed += 1
                        continue
                    seen = True
                keep.append(x)
            if removed:
                bb.instructions = keep
    nc.finalize()
    return nc


# ---------------- entry point ----------------
def kernel(image_feature, post_trans, post_rots, intrinsics, extrinsics,
           frustum, bev_res, bev_start_pos):
    from concourse.bass_utils import run_bass_kernel_spmd
    import os

    rank, mask = _compute_ranks(frustum, post_trans, post_rots, intrinsics,
                                extrinsics, bev_res, bev_start_pos)
    feats16 = np.asarray(image_feature, np.float32).reshape(
        B, NP_SAMPLE, C).astype(np.float16)
    cores, NB, Kb, base, NC = _plan_cores(rank, mask)

    ident = np.eye(128, dtype=np.float16)
    in_maps = [
        {"table": _build_table(c, feats16[c['sample']], NB, Kb, base, NC),
         "ident": ident}
        for c in cores
    ]

    key = (NB, tuple(int(k) for k in Kb), NC)
    if key not in _compiled:
        _compiled[key] = _build_kernel(NB, Kb, NC)
    nc = _compiled[key]

    trace = bool(int(os.environ.get("BEV_TRACE", "0")))
    res = run_bass_kernel_spmd(nc, in_maps, core_ids=list(range(NCORES)),
                               trace=trace,
                               trace_cores=[0] if trace else None)
    if trace and res.exec_time_ns is not None:
        print(f"HW exec time: {res.exec_time_ns} ns")
        kernel.last_exec_time_ns = res.exec_time_ns
        kernel.last_results = res

    grid = np.zeros((B, NBINS, C), np.float32)
    for ci, core in enumerate(cores):
        o = np.asarray(res.results[ci]["out"], np.float32)
        n = len(core['cnt'])
        if n:
            np.add.at(grid[core['sample']], core['rank'], o[:n])
    return np.ascontiguousarray(
        grid.reshape(B, X, Y, C).transpose(0, 3, 1, 2))
